# revision 67
# baseline (speedup 1.0000x reference)
"""Distributed multi-head causal attention for 8 TRN2 NeuronCores.

Problem: residual [2, 2048, 2048] f32 -> (residual, attn_out [2, 2048, 2048])
  q/k/v = residual @ W_{Q,K,V} + b  (16 heads, d_head 128)
  scores = q k^T / sqrt(128), causal mask, softmax
  out = (pattern @ v) @ W_O + b_O

Sharding: tensor-parallel over heads. Core c computes QKV projections and
attention for heads 2c, 2c+1 over both batches, producing z^T (the
pre-output-projection activations). Two 8-core AllToAlls (one per local
head, so the first can fly while the second head's attention still runs)
redistribute z^T from head-sharded to position-sharded: shard j covers
positions [512*j, 512*(j+1)) of the flattened [batch*seq] axis. After the
A2A each core holds all 16 heads for its own 512 positions and computes the
output projection for just those rows. The host concatenates the 8 shards.

All matmuls in bf16 (inputs pre-cast and pre-packed on host so every load is
one large contiguous DMA), accumulation f32 in PSUM. The residual is fed
pre-transposed (X^T) so every matmul's operands are in natural layout
(contraction dim on partitions):
  Q^T/K^T [dh, pos] = W^T X^T    (lhsT = W [model, dh], rhs = X^T)
  V [pos, dh*hpc]   = X W_V      (lhsT = X^T tile,      rhs = W_V heads)
  S^T [k, q]        = K Q^T      (lhsT = K^T tile,      rhs = Q^T)
  z^T [dh, q]       = V^T P^T    (lhsT = V half||ones,  rhs = P^T = exp(S^T))
  out [pos, m]      = z W_O      (lhsT = z^T tile,      rhs = W_O)

V is stored as [64-column half | ones] pairs so each M=65 AV matmul's 65th
output row is the softmax denominator (no separate sum matmul). Attention
runs as a software pipeline over (head, batch, q-chunk) chunks: the scores
pass of chunk i interleaves at k-tile granularity with the AV pass of chunk
i-1 (two stationary streams pipeline through the PE's two weight buffers,
and the scalar-engine exps get slack between score matmuls), while the
normalize/ship stage of chunk i-2 leads each iteration with a rank-1 PE
broadcast of the softmax sums.
"""

import numpy as np
import ml_dtypes

import concourse.bass as bass
import concourse.tile as tile
from concourse import bacc, mybir
from concourse.bass_utils import run_bass_kernel_spmd
from concourse.tile_rust import add_dep_helper

BF16 = mybir.dt.bfloat16
F32 = mybir.dt.float32
NP_BF16 = ml_dtypes.bfloat16

FULL = dict(n_heads=16, d_model=2048, d_head=128, batch=2, seq=2048, n_cores=8)
ATTN_SCALE = float(np.sqrt(128.0))


def _derived(cfg):
    d = dict(cfg)
    d["hpc"] = d["n_heads"] // d["n_cores"]             # heads per core
    d["rows"] = d["batch"] * d["seq"] // d["n_cores"]   # out rows per core
    d["qc_size"] = d["rows"]                            # q-chunk == A2A shard
    assert d["qc_size"] <= 512
    d["n_qc"] = d["seq"] // d["qc_size"]                # q chunks per batch
    d["n_kb"] = d["seq"] // 128                         # k blocks per batch
    d["n_mb"] = d["d_model"] // 128                     # model-dim blocks
    d["n_dg"] = d["qc_size"] // 128                     # diag offsets per chunk
    d["n_mc"] = d["d_model"] // 512                     # out m-chunks
    d["n_pb"] = d["rows"] // 128                        # out pos-blocks
    assert d["n_qc"] * d["batch"] == d["n_cores"]
    assert d["d_head"] == 128
    return d


def build_graph(cfg=FULL, enable_asserts=False):
    c = _derived(cfg)
    hpc, QC = c["hpc"], c["qc_size"]
    n_qc, n_kb, n_mb, n_dg = c["n_qc"], c["n_kb"], c["n_mb"], c["n_dg"]
    n_mc, n_pb, rows = c["n_mc"], c["n_pb"], c["rows"]
    n_heads, d_model, seq = c["n_heads"], c["d_model"], c["seq"]
    batch, n_cores = c["batch"], c["n_cores"]
    dpb = QC // 128
    MC = 512

    nc = bacc.Bacc("TRN2", target_bir_lowering=False, debug=False,
                   enable_asserts=enable_asserts, num_devices=n_cores)

    # all inputs pre-packed on host into [128, ...] partition-major layouts
    xt_d = nc.dram_tensor("xt", [128, batch, n_qc, n_mb, QC], BF16,
                          kind="ExternalInput")
    wq_d = nc.dram_tensor("wq", [128, hpc, n_mb, 128], BF16, kind="ExternalInput")
    wk_d = nc.dram_tensor("wk", [128, hpc, n_mb, 128], BF16, kind="ExternalInput")
    wv_d = nc.dram_tensor("wv", [128, n_mb, hpc * 128], BF16, kind="ExternalInput")
    wo_d = nc.dram_tensor("wo", [128, n_heads, d_model], BF16, kind="ExternalInput")
    bq_d = nc.dram_tensor("bq", [128, hpc], F32, kind="ExternalInput")
    bk_d = nc.dram_tensor("bk", [128, hpc], F32, kind="ExternalInput")
    bv_d = nc.dram_tensor("bv", [hpc * 128], F32, kind="ExternalInput")
    mk_d = nc.dram_tensor("mk", [128, n_dg, QC], BF16, kind="ExternalInput")
    out_d = nc.dram_tensor("out", [rows, d_model], F32, kind="ExternalOutput")

    rg = [list(range(n_cores))]
    Exp = mybir.ActivationFunctionType.Exp

    with tile.TileContext(nc) as tc:
        with (
            tc.tile_pool(name="stat", bufs=1) as stat,
            tc.tile_pool(name="xin", bufs=2) as xin,
            tc.tile_pool(name="work", bufs=3) as work,
            tc.tile_pool(name="ps", bufs=2, space="PSUM") as ps,
            tc.tile_pool(name="dram", bufs=1, space="DRAM") as dram,
        ):
            wq_sb = stat.tile([128, hpc, n_mb, 128], BF16)
            wk_sb = stat.tile([128, hpc, n_mb, 128], BF16)
            wv_sb = stat.tile([128, n_mb, hpc * 128], BF16, tag="wvzf")
            qt_sb = stat.tile([128, batch, hpc, seq], BF16)
            kt_sb = stat.tile([128, batch, hpc, seq], BF16)
            # V stored as [64-col half | ones] pairs so the AV matmul's 65th
            # output row is the softmax denominator (no separate sum matmul)
            v_sb = stat.tile([128, batch, n_kb, hpc, 2, 65], BF16)
            bq_sb = stat.tile([128, hpc], F32)
            bk_sb = stat.tile([128, hpc], F32)
            vb_sb = stat.tile([128, hpc * 128], F32)
            mk_sb = stat.tile([128, n_dg, QC], BF16)
            onesrow_sb = stat.tile([128, 128], F32)

            a2a_in = [dram.tile([n_cores, 128, rows], BF16, name=f"a2ai{h}")
                      for h in range(hpc)]
            a2a_out = [dram.tile([n_cores, 128, rows], BF16, name=f"a2ao{h}")
                       for h in range(hpc)]

            # constant loads: big weights on the scalar HWDGE queue, small
            # consts on gpsimd, xt chunks stream on sync inside phase 1
            for mb in range(n_mb):
                nc.scalar.dma_start(out=wq_sb[:, 0, mb], in_=wq_d[:, 0, mb])
            for h in range(1, hpc):
                nc.scalar.dma_start(out=wq_sb[:, h], in_=wq_d[:, h])
            nc.scalar.dma_start(out=wv_sb, in_=wv_d[:])
            nc.gpsimd.dma_start(out=bq_sb, in_=bq_d[:])
            nc.gpsimd.dma_start(out=bk_sb, in_=bk_d[:])
            bv_ap = bv_d.ap()
            nc.gpsimd.dma_start(
                out=vb_sb,
                in_=bass.AP(tensor=bv_ap.tensor, offset=bv_ap.offset,
                            ap=[[0, 128]] + list(bv_ap.ap)))
            nc.gpsimd.dma_start(out=mk_sb, in_=mk_d[:])
            nc.vector.memset(onesrow_sb, 1.0)
            for b in range(batch):
                for kb in range(n_kb):
                    nc.vector.memset(v_sb[:, b, kb, :, :, 64:65], 1.0)

            # ---- phase 1: QKV projections (streaming X^T per q-chunk) ----
            for b in range(batch):
                for qc in range(n_qc):
                    ql = slice(qc * QC, (qc + 1) * QC)
                    xt_c = xin.tile([128, n_mb, QC], BF16, tag="xt",
                                    name=f"xt{b}_{qc}")
                    if b == 0 and qc == 0:
                        # per-mb loads for the first chunk: the leading
                        # projection matmuls start as soon as the first 64KB
                        # slices land on the cold queues; K weights follow,
                        # arriving just before the first K psum
                        for mb in range(n_mb):
                            nc.sync.dma_start(out=xt_c[:, mb],
                                              in_=xt_d[:, b, qc, mb])
                        nc.sync.dma_start(out=wk_sb, in_=wk_d[:])
                    else:
                        nc.sync.dma_start(out=xt_c, in_=xt_d[:, b, qc])
                    for h in range(hpc):
                        for (w_sb, b_sb, dst) in ((wq_sb, bq_sb, qt_sb),
                                                  (wk_sb, bk_sb, kt_sb)):
                            pp = ps.tile([128, QC], F32, tag="A",
                                         name=f"psp{b}_{qc}_{h}")
                            for mb in range(n_mb):
                                nc.tensor.matmul(pp, w_sb[:, h, mb, :],
                                                 xt_c[:, mb, :],
                                                 start=(mb == 0),
                                                 stop=(mb == n_mb - 1))
                            nc.vector.tensor_scalar_add(dst[:, b, h, ql], pp,
                                                        b_sb[:, h:h + 1])
                    for pb4 in range(dpb):
                        pb = qc * dpb + pb4
                        pp = ps.tile([128, hpc * 128], F32, tag="A",
                                     name=f"psv{b}_{pb}")
                        for mb in range(n_mb):
                            nc.tensor.matmul(
                                pp, xt_c[:, mb, pb4 * 128:(pb4 + 1) * 128],
                                wv_sb[:, mb, :],
                                start=(mb == 0), stop=(mb == n_mb - 1))
                        nc.vector.tensor_add(
                            v_sb[:, b, pb, :, :, 0:64],
                            pp.rearrange("p (h t d) -> p h t d", h=hpc, t=2),
                            vb_sb.rearrange("p (h t d) -> p h t d", h=hpc, t=2))

            wo_tiles = {}

            # ---- phase 2: attention ----
            # Largest q-chunks first within each head so the last chunks'
            # normalize/ship chains hide under bigger earlier compute. The
            # scores pass of chunk i is interleaved at k-tile granularity
            # with the AV pass of chunk i-1: only two stationary streams
            # (K tiles, V tiles) alternate, which the PE's two weight
            # buffers pipeline cleanly, and the scalar-engine exps get ~3x
            # the slack per score-psum slot (no pass-A burst throttling).
            chunks = [(h, b, qc) for h in range(hpc) for b in range(batch)
                      for qc in reversed(range(n_qc))]
            panels = {}
            chain = {}
            last_exp = [None]

            def emit_A_tile(ci, h, b, qc, kb, panel):
                dg = kb - qc * dpb
                off = 128 * dg if dg > 0 else 0
                np_ = QC - off
                st_tag, st_bufs = (("st", 3), ("A", 2))[kb % 2]
                st = ps.tile([128, QC], F32, tag=st_tag, bufs=st_bufs,
                             name=f"st{ci}_{kb}")
                nc.tensor.matmul(st[:, :np_],
                                 kt_sb[:, b, h, kb * 128:(kb + 1) * 128],
                                 qt_sb[:, b, h, qc * QC + off:(qc + 1) * QC],
                                 start=True, stop=True)
                last_exp[0] = nc.scalar.activation(panel[:, kb, off:],
                                                   st[:, :np_], Exp)
                if dg >= 0:
                    band = slice(off, off + 128)
                    nc.vector.tensor_mul(panel[:, kb, band],
                                         panel[:, kb, band],
                                         mk_sb[:, dg, band])

            def emit_iter(a_args, b_args):
                n_a = n_b = 0
                if a_args is not None:
                    ci_a, h_a, b_a, qc_a = a_args
                    n_a = (qc_a + 1) * dpb
                    panel_a = work.tile([128, n_kb, QC], BF16, tag="pt",
                                        bufs=2, name=f"pt{ci_a}")
                    panels[ci_a] = panel_a
                if b_args is not None:
                    ci_b, h_b, b_b, qc_b = b_args
                    n_b = (qc_b + 1) * dpb
                    panel_b = panels.pop(ci_b)
                    zps = [ps.tile([65, QC], F32, tag="z", bufs=3,
                                   name=f"zt{ci_b}_{half}")
                           for half in (0, 1)]
                for kb in range(max(n_a, n_b)):
                    if kb < n_a:
                        emit_A_tile(ci_a, h_a, b_a, qc_a, kb, panel_a)
                    if kb < n_b:
                        dg = kb - qc_b * dpb
                        off = 128 * dg if dg > 0 else 0
                        for half, zp in ((0, zps[0]), (1, zps[1])):
                            nc.tensor.matmul(zp[:, off:],
                                             v_sb[:, b_b, kb, h_b, half, :],
                                             panel_b[:, kb, off:],
                                             start=(kb == 0),
                                             stop=(kb == n_b - 1),
                                             skip_group_check=True)
                if b_args is None:
                    return
                zta, ztb = zps
                # drain PSUM fast: s row + both z halves to SBUF (3 quick
                # DVE ops) so the z banks recycle ahead of the ship below
                ssb = work.tile([128, QC], F32, tag="ssb", bufs=3,
                                name=f"ssb{ci_b}")
                nc.vector.tensor_copy(ssb[64:65, :], zta[64:65, :])
                zsb = []
                for half, zp in ((0, zta), (1, ztb)):
                    zs = work.tile([64, QC], BF16, tag=f"zsb{half}", bufs=3,
                                   name=f"zsb{ci_b}_{half}")
                    nc.vector.tensor_copy(zs, zp[0:64, :])
                    zsb.append(zs)
                chain[ci_b] = (ssb, zsb, h_b, b_b, qc_b)

            def ship(ci):
                # one chunk behind its AV pass: broadcast s to all partitions
                # with a rank-1 PE matmul (deterministically paced by the PE
                # stream, no DRAM bounce, no gpsimd hop), then 1/s and scale
                ssb, zsb, h, b, qc = chain.pop(ci)
                rb_ps = ps.tile([128, QC], F32, tag="z", bufs=3,
                                name=f"rbp{ci}")
                nc.tensor.matmul(rb_ps, onesrow_sb[64:65, :], ssb[64:65, :],
                                 start=True, stop=True)
                rb = work.tile([128, QC], F32, tag="rb", bufs=3, name=f"rb{ci}")
                nc.vector.reciprocal_approx_fast(out=rb, in_=rb_ps)
                shard = b * n_qc + qc
                for half, zs in ((0, zsb[0]), (1, zsb[1])):
                    nc.vector.tensor_mul(zs, zs, rb[0:64, :])
                    nc.sync.dma_start(
                        out=a2a_in[h][shard][half * 64:(half + 1) * 64, :],
                        in_=zs)

            zf_sb = stat.tile([128, n_heads, rows], BF16, tag="wvzf")

            def emit_cc(h):
                nc.gpsimd.collective_compute(
                    "AllToAll", mybir.AluOpType.bypass, replica_groups=rg,
                    ins=[a2a_in[h].opt()], outs=[a2a_out[h].opt()])

            def load_zf(h):
                # scalar HWDGE queue: idle after phase 2's exps. Explicitly
                # pinned after the final exp so the scheduler cannot hoist the
                # collective-completion wait into the middle of phase 2.
                for i in range(n_cores):
                    d = nc.scalar.dma_start(out=zf_sb[:, i * hpc + h, :],
                                            in_=a2a_out[h][i])
                    if last_exp[0] is not None:
                        add_dep_helper(d.ins, last_exp[0].ins,
                                       reason="zf load after phase-2 exps")

            per_h = len(chunks) // hpc

            def do_ship(ci, h):
                ship(ci)
                if ci % per_h == per_h - 1:
                    emit_cc(h)

            prev = None
            for ci, (h, b, qc) in enumerate(chunks):
                if ci >= 2:
                    do_ship(ci - 2, chunks[ci - 2][0])
                emit_iter((ci, h, b, qc), prev)
                prev = (ci, h, b, qc)
            n = len(chunks)
            if n >= 2:
                do_ship(n - 2, chunks[n - 2][0])
            emit_iter(None, prev)
            do_ship(n - 1, chunks[n - 1][0])
            # scalar-queue order at phase-2 end: zf evens (cc0 long done),
            # then W_O chunks 0/1 into the released qt/kt slots, then zf odds
            # (which wait on cc1). All pinned after the exps so the scheduler
            # cannot hoist any of these waits into phase 2.
            load_zf(0)
            for mc, tg in zip(range(min(2, n_mc)), ("qt_sb", "kt_sb")):
                t = stat.tile([128, n_heads, MC], BF16, tag=tg, name=f"wo{mc}")
                d = nc.scalar.dma_start(out=t,
                                        in_=wo_d[:, :, mc * MC:(mc + 1) * MC])
                if last_exp[0] is not None:
                    add_dep_helper(d.ins, last_exp[0].ins,
                                   reason="wo load after phase-2 exps")
                wo_tiles[mc] = t
            for h in range(1, hpc):
                load_zf(h)
            # remaining W_O chunks reuse the idle xt slots, on the gpsimd
            # queue right after the collective triggers
            for mc in range(2, n_mc):
                t = xin.tile([128, n_heads, MC], BF16, tag="xt", name=f"wo{mc}")
                nc.gpsimd.dma_start(
                    out=t, in_=wo_d[:, :, mc * MC:(mc + 1) * MC])
                wo_tiles[mc] = t

            # ---- phase 3: output projection, two passes ----
            # Pass E runs the first-A2A heads for ALL output tiles (~33us of
            # PE work gated only on cc0) staged to SBUF in bf16; pass O adds
            # the remaining heads once cc1's zf tiles have long arrived.
            heads_of = {h: [i * hpc + h for i in range(n_cores)]
                        for h in range(hpc)}
            late_heads = [g for h in range(1, hpc) for g in heads_of[h]]
            osbe = stat.tile([128, n_mc * n_pb, MC], BF16)
            tiles3 = [(mc, pb) for mc in range(n_mc) for pb in range(n_pb)]
            for n, (mc, pb) in enumerate(tiles3):
                tg, bf = (("A", 2), ("st", 3))[n % 2]
                pp = ps.tile([128, MC], F32, tag=tg, bufs=bf,
                             name=f"pse{mc}_{pb}")
                for j, g in enumerate(heads_of[0]):
                    nc.tensor.matmul(pp, zf_sb[:, g, pb * 128:(pb + 1) * 128],
                                     wo_tiles[mc][:, g, :],
                                     start=(j == 0),
                                     stop=(j == len(heads_of[0]) - 1))
                nc.vector.tensor_copy(osbe[:, n, :], pp)
            for n, (mc, pb) in enumerate(tiles3):
                tg, bf = (("A", 2), ("st", 3))[n % 2]
                pp = ps.tile([128, MC], F32, tag=tg, bufs=bf,
                             name=f"pso{mc}_{pb}")
                for j, g in enumerate(late_heads):
                    nc.tensor.matmul(pp, zf_sb[:, g, pb * 128:(pb + 1) * 128],
                                     wo_tiles[mc][:, g, :],
                                     start=(j == 0),
                                     stop=(j == len(late_heads) - 1))
                osb = work.tile([128, MC], F32, tag="osb", bufs=2,
                                name=f"osb{mc}_{pb}")
                nc.vector.tensor_add(osb, pp, osbe[:, n, :])
                ml = slice(mc * MC, (mc + 1) * MC)
                nc.sync.dma_start(out=out_d[pb * 128:(pb + 1) * 128, ml],
                                  in_=osb)

    nc.compile()
    return nc


def make_in_maps(inputs, cfg=FULL):
    c = _derived(cfg)
    hpc, QC = c["hpc"], c["qc_size"]
    n_mb, n_dg = c["n_mb"], c["n_dg"]
    d_model, seq, batch = c["d_model"], c["seq"], c["batch"]
    residual = np.asarray(inputs["residual"], np.float32)
    W_Q = np.asarray(inputs["W_Q"], np.float32)
    W_K = np.asarray(inputs["W_K"], np.float32)
    W_V = np.asarray(inputs["W_V"], np.float32)
    W_O = np.asarray(inputs["W_O"], np.float32)
    b_Q = np.asarray(inputs["b_Q"], np.float32)
    b_K = np.asarray(inputs["b_K"], np.float32)
    b_V = np.asarray(inputs["b_V"], np.float32)
    scale = 1.0 / ATTN_SCALE

    # X^T packed per q-chunk [128, batch, n_qc, n_mb, QC]:
    # [p, b, qc, mb, s'] = residual[b, qc*QC+s', mb*128+p]
    n_qc = c["n_qc"]
    xt = np.ascontiguousarray(
        residual.reshape(batch, n_qc, QC, n_mb, 128).transpose(4, 0, 1, 3, 2)
    ).astype(NP_BF16)
    # W_O packed [128, n_heads, d_model]
    wo = np.ascontiguousarray(
        W_O.transpose(1, 0, 2)).astype(NP_BF16)
    # causal {0,1} masks packed [128, n_dg, QC]
    masks = np.zeros((128, n_dg, QC), np.float32)
    pk = np.arange(128)[:, None]
    fq = np.arange(QC)[None, :]
    for dg in range(n_dg):
        masks[:, dg, :] = (fq >= pk + 128 * dg).astype(np.float32)
    masks = masks.astype(NP_BF16)

    in_maps = []
    for core in range(c["n_cores"]):
        hs = slice(core * hpc, (core + 1) * hpc)
        # [128, hpc, n_mb, 128]: [p, h, mb, d] = W[h, mb*128+p, d]
        wq = np.ascontiguousarray(
            (W_Q[hs] * scale).reshape(hpc, n_mb, 128, 128).transpose(2, 0, 1, 3)
        ).astype(NP_BF16)
        wk = np.ascontiguousarray(
            W_K[hs].reshape(hpc, n_mb, 128, 128).transpose(2, 0, 1, 3)
        ).astype(NP_BF16)
        # [128, n_mb, hpc*128]: [p, mb, (h d)] = W_V[h, mb*128+p, d]
        wv = np.ascontiguousarray(
            W_V[hs].reshape(hpc, n_mb, 128, 128).transpose(2, 1, 0, 3)
            .reshape(128, n_mb, hpc * 128)).astype(NP_BF16)
        bq = np.ascontiguousarray((b_Q[hs] * scale).T).astype(np.float32)
        bk = np.ascontiguousarray(b_K[hs].T).astype(np.float32)
        bv = np.ascontiguousarray(b_V[hs].reshape(hpc * 128)).astype(np.float32)
        in_maps.append({
            "xt": xt, "wq": wq, "wk": wk, "wv": wv, "wo": wo,
            "bq": bq, "bk": bk, "bv": bv, "mk": masks,
        })
    return in_maps


def assemble_output(inputs, shards, cfg=FULL):
    c = _derived(cfg)
    residual = np.asarray(inputs["residual"], np.float32)
    b_O = np.asarray(inputs["b_O"], np.float32)
    out = np.concatenate([np.asarray(s, np.float32) for s in shards], axis=0)
    out = out.reshape(c["batch"], c["seq"], c["d_model"]) + b_O
    return residual, out.astype(np.float32)


_NC_CACHE = {}


def _get_nc():
    if "nc" not in _NC_CACHE:
        _NC_CACHE["nc"] = build_graph(FULL)
    return _NC_CACHE["nc"]


def run(inputs, trace=False):
    nc = _get_nc()
    in_maps = make_in_maps(inputs, FULL)
    try:
        res = run_bass_kernel_spmd(nc, in_maps, list(range(FULL["n_cores"])),
                                   trace=trace)
    except Exception:
        # a previous bad run can leave the remote device wedged for one
        # attempt; give it a moment and retry once
        import time
        time.sleep(60)
        res = run_bass_kernel_spmd(nc, in_maps, list(range(FULL["n_cores"])),
                                   trace=trace)
    shards = [res.results[i]["out"] for i in range(FULL["n_cores"])]
    residual, out = assemble_output(inputs, shards, FULL)
    return (residual, out), res


def kernel(**inputs):
    (residual, out), _ = run(inputs, trace=False)
    return (residual, out)


# revision 69
# speedup vs baseline: 1.1210x; 1.1210x over previous
"""Distributed multi-head causal attention for 8 TRN2 NeuronCores.

Problem: residual [2, 2048, 2048] f32 -> (residual, attn_out [2, 2048, 2048])
  q/k/v = residual @ W_{Q,K,V} + b  (16 heads, d_head 128)
  scores = q k^T / sqrt(128), causal mask, softmax
  out = (pattern @ v) @ W_O + b_O

Sharding: tensor-parallel over heads. Core c computes QKV projections and
attention for heads 2c, 2c+1 over both batches, producing z^T (the
pre-output-projection activations). Two 8-core AllToAlls (one per local
head, so the first can fly while the second head's attention still runs)
redistribute z^T from head-sharded to position-sharded: shard j covers
positions [512*j, 512*(j+1)) of the flattened [batch*seq] axis. After the
A2A each core holds all 16 heads for its own 512 positions and computes the
output projection for just those rows. The host concatenates the 8 shards.

All matmuls in bf16 (inputs pre-cast and pre-packed on host so every load is
one large contiguous DMA), accumulation f32 in PSUM. The residual is fed
pre-transposed (X^T) so every matmul's operands are in natural layout
(contraction dim on partitions):
  Q^T/K^T [dh, pos] = W^T X^T    (lhsT = W [model, dh], rhs = X^T)
  V [pos, dh*hpc]   = X W_V      (lhsT = X^T tile,      rhs = W_V heads)
  S^T [k, q]        = K Q^T      (lhsT = K^T tile,      rhs = Q^T)
  z^T [dh, q]       = V^T P^T    (lhsT = V half||ones,  rhs = P^T = exp(S^T))
  out [pos, m]      = z W_O      (lhsT = z^T tile,      rhs = W_O)

V is stored as [64-column half | ones] pairs so each M=65 AV matmul's 65th
output row is the softmax denominator (no separate sum matmul). Attention
runs as a software pipeline over (head, batch, q-chunk) chunks: the scores
pass of chunk i interleaves at k-tile granularity with the AV pass of chunk
i-1 (two stationary streams pipeline through the PE's two weight buffers,
and the scalar-engine exps get slack between score matmuls), while the
normalize/ship stage of chunk i-2 leads each iteration with a rank-1 PE
broadcast of the softmax sums.
"""

import numpy as np
import ml_dtypes

import concourse.bass as bass
import concourse.tile as tile
from concourse import bacc, mybir
from concourse.bass_utils import run_bass_kernel_spmd
from concourse.tile_rust import add_dep_helper

BF16 = mybir.dt.bfloat16
F32 = mybir.dt.float32
NP_BF16 = ml_dtypes.bfloat16

FULL = dict(n_heads=16, d_model=2048, d_head=128, batch=2, seq=2048, n_cores=8)
ATTN_SCALE = float(np.sqrt(128.0))


def _derived(cfg):
    d = dict(cfg)
    d["hpc"] = d["n_heads"] // d["n_cores"]             # heads per core
    d["rows"] = d["batch"] * d["seq"] // d["n_cores"]   # out rows per core
    d["qc_size"] = d["rows"]                            # q-chunk == A2A shard
    assert d["qc_size"] <= 512
    d["n_qc"] = d["seq"] // d["qc_size"]                # q chunks per batch
    d["n_kb"] = d["seq"] // 128                         # k blocks per batch
    d["n_mb"] = d["d_model"] // 128                     # model-dim blocks
    d["n_dg"] = d["qc_size"] // 128                     # diag offsets per chunk
    d["n_mc"] = d["d_model"] // 512                     # out m-chunks
    d["n_pb"] = d["rows"] // 128                        # out pos-blocks
    assert d["n_qc"] * d["batch"] == d["n_cores"]
    assert d["d_head"] == 128
    return d


def build_graph(cfg=FULL, enable_asserts=False):
    c = _derived(cfg)
    hpc, QC = c["hpc"], c["qc_size"]
    n_qc, n_kb, n_mb, n_dg = c["n_qc"], c["n_kb"], c["n_mb"], c["n_dg"]
    n_mc, n_pb, rows = c["n_mc"], c["n_pb"], c["rows"]
    n_heads, d_model, seq = c["n_heads"], c["d_model"], c["seq"]
    batch, n_cores = c["batch"], c["n_cores"]
    dpb = QC // 128
    MC = 512

    nc = bacc.Bacc("TRN2", target_bir_lowering=False, debug=False,
                   enable_asserts=enable_asserts, num_devices=n_cores)

    # all inputs pre-packed on host into [128, ...] partition-major layouts
    xt_d = nc.dram_tensor("xt", [128, batch, n_qc, n_mb, QC], BF16,
                          kind="ExternalInput")
    wq_d = nc.dram_tensor("wq", [128, hpc, n_mb, 128], BF16, kind="ExternalInput")
    wk_d = nc.dram_tensor("wk", [128, hpc, n_mb, 128], BF16, kind="ExternalInput")
    wv_d = nc.dram_tensor("wv", [128, n_mb, hpc * 128], BF16, kind="ExternalInput")
    wo_d = nc.dram_tensor("wo", [128, n_heads, d_model], BF16, kind="ExternalInput")
    bq_d = nc.dram_tensor("bq", [128, hpc], F32, kind="ExternalInput")
    bk_d = nc.dram_tensor("bk", [128, hpc], F32, kind="ExternalInput")
    bv_d = nc.dram_tensor("bv", [hpc * 128], F32, kind="ExternalInput")
    mk_d = nc.dram_tensor("mk", [128, n_dg, QC], BF16, kind="ExternalInput")
    out_d = nc.dram_tensor("out", [rows, d_model], F32, kind="ExternalOutput")

    rg = [list(range(n_cores))]
    Exp = mybir.ActivationFunctionType.Exp

    with tile.TileContext(nc) as tc:
        with (
            tc.tile_pool(name="stat", bufs=1) as stat,
            tc.tile_pool(name="xin", bufs=2) as xin,
            tc.tile_pool(name="work", bufs=3) as work,
            tc.tile_pool(name="ps", bufs=2, space="PSUM") as ps,
            tc.tile_pool(name="dram", bufs=1, space="DRAM") as dram,
        ):
            wq_sb = stat.tile([128, hpc, n_mb, 128], BF16)
            wk_sb = stat.tile([128, hpc, n_mb, 128], BF16)
            wv_sb = stat.tile([128, n_mb, hpc * 128], BF16, tag="wvzf")
            qt_sb = stat.tile([128, batch, hpc, seq], BF16)
            kt_sb = stat.tile([128, batch, hpc, seq], BF16)
            # V stored as [64-col half | ones] pairs so the AV matmul's 65th
            # output row is the softmax denominator (no separate sum matmul)
            v_sb = stat.tile([128, batch, n_kb, hpc, 2, 65], BF16)
            bq_sb = stat.tile([128, hpc], F32)
            bk_sb = stat.tile([128, hpc], F32)
            vb_sb = stat.tile([128, hpc * 128], F32)
            mk_sb = stat.tile([128, n_dg, QC], BF16)
            onesrow_sb = stat.tile([128, 128], F32)

            a2a_in = [dram.tile([n_cores, 128, rows], BF16, name=f"a2ai{h}")
                      for h in range(hpc)]
            a2a_out = [dram.tile([n_cores, 128, rows], BF16, name=f"a2ao{h}")
                       for h in range(hpc)]

            # constant loads: big weights on the scalar HWDGE queue, small
            # consts on gpsimd, xt chunks stream on sync inside phase 1
            for mb in range(n_mb):
                nc.scalar.dma_start(out=wq_sb[:, 0, mb], in_=wq_d[:, 0, mb])
            for h in range(1, hpc):
                nc.scalar.dma_start(out=wq_sb[:, h], in_=wq_d[:, h])
            nc.scalar.dma_start(out=wv_sb, in_=wv_d[:])
            nc.gpsimd.dma_start(out=bq_sb, in_=bq_d[:])
            nc.gpsimd.dma_start(out=bk_sb, in_=bk_d[:])
            bv_ap = bv_d.ap()
            nc.gpsimd.dma_start(
                out=vb_sb,
                in_=bass.AP(tensor=bv_ap.tensor, offset=bv_ap.offset,
                            ap=[[0, 128]] + list(bv_ap.ap)))
            nc.gpsimd.dma_start(out=mk_sb, in_=mk_d[:])
            nc.vector.memset(onesrow_sb, 1.0)
            for b in range(batch):
                for kb in range(n_kb):
                    nc.vector.memset(v_sb[:, b, kb, :, :, 64:65], 1.0)

            # ---- phase 1: QKV projections (streaming X^T per q-chunk) ----
            for b in range(batch):
                for qc in range(n_qc):
                    ql = slice(qc * QC, (qc + 1) * QC)
                    xt_c = xin.tile([128, n_mb, QC], BF16, tag="xt",
                                    name=f"xt{b}_{qc}")
                    if b == 0 and qc == 0:
                        # per-mb loads for the first chunk: the leading
                        # projection matmuls start as soon as the first 64KB
                        # slices land on the cold queues; K weights follow,
                        # arriving just before the first K psum
                        for mb in range(n_mb):
                            nc.sync.dma_start(out=xt_c[:, mb],
                                              in_=xt_d[:, b, qc, mb])
                        nc.sync.dma_start(out=wk_sb, in_=wk_d[:])
                    else:
                        nc.sync.dma_start(out=xt_c, in_=xt_d[:, b, qc])
                    for h in range(hpc):
                        for (w_sb, b_sb, dst) in ((wq_sb, bq_sb, qt_sb),
                                                  (wk_sb, bk_sb, kt_sb)):
                            pp = ps.tile([128, QC], F32, tag="A",
                                         name=f"psp{b}_{qc}_{h}")
                            for mb in range(n_mb):
                                nc.tensor.matmul(pp, w_sb[:, h, mb, :],
                                                 xt_c[:, mb, :],
                                                 start=(mb == 0),
                                                 stop=(mb == n_mb - 1))
                            nc.vector.tensor_scalar_add(dst[:, b, h, ql], pp,
                                                        b_sb[:, h:h + 1])
                    for pb4 in range(dpb):
                        pb = qc * dpb + pb4
                        pp = ps.tile([128, hpc * 128], F32, tag="A",
                                     name=f"psv{b}_{pb}")
                        for mb in range(n_mb):
                            nc.tensor.matmul(
                                pp, xt_c[:, mb, pb4 * 128:(pb4 + 1) * 128],
                                wv_sb[:, mb, :],
                                start=(mb == 0), stop=(mb == n_mb - 1))
                        nc.vector.tensor_add(
                            v_sb[:, b, pb, :, :, 0:64],
                            pp.rearrange("p (h t d) -> p h t d", h=hpc, t=2),
                            vb_sb.rearrange("p (h t d) -> p h t d", h=hpc, t=2))

            wo_tiles = {}

            # ---- phase 2: attention ----
            # Largest q-chunks first within each head so the last chunks'
            # normalize/ship chains hide under bigger earlier compute. The
            # scores pass of chunk i is interleaved at k-tile granularity
            # with the AV pass of chunk i-1: only two stationary streams
            # (K tiles, V tiles) alternate, which the PE's two weight
            # buffers pipeline cleanly, and the scalar-engine exps get ~3x
            # the slack per score-psum slot (no pass-A burst throttling).
            chunks = [(h, b, qc) for h in range(hpc) for b in range(batch)
                      for qc in reversed(range(n_qc))]
            panels = {}
            chain = {}
            last_exp = [None]

            def emit_A_tile(ci, h, b, qc, kb, panel):
                dg = kb - qc * dpb
                off = 128 * dg if dg > 0 else 0
                np_ = QC - off
                st_tag, st_bufs = (("st", 3), ("A", 2))[kb % 2]
                st = ps.tile([128, QC], F32, tag=st_tag, bufs=st_bufs,
                             name=f"st{ci}_{kb}")
                nc.tensor.matmul(st[:, :np_],
                                 kt_sb[:, b, h, kb * 128:(kb + 1) * 128],
                                 qt_sb[:, b, h, qc * QC + off:(qc + 1) * QC],
                                 start=True, stop=True)
                last_exp[0] = nc.scalar.activation(panel[:, kb, off:],
                                                   st[:, :np_], Exp)
                if dg >= 0:
                    band = slice(off, off + 128)
                    nc.vector.tensor_mul(panel[:, kb, band],
                                         panel[:, kb, band],
                                         mk_sb[:, dg, band])

            def emit_iter(a_args, b_args):
                n_a = n_b = 0
                if a_args is not None:
                    ci_a, h_a, b_a, qc_a = a_args
                    n_a = (qc_a + 1) * dpb
                    panel_a = work.tile([128, n_kb, QC], BF16, tag="pt",
                                        bufs=2, name=f"pt{ci_a}")
                    panels[ci_a] = panel_a
                if b_args is not None:
                    ci_b, h_b, b_b, qc_b = b_args
                    n_b = (qc_b + 1) * dpb
                    panel_b = panels.pop(ci_b)
                    zps = [ps.tile([65, QC], F32, tag="z", bufs=3,
                                   name=f"zt{ci_b}_{half}")
                           for half in (0, 1)]
                for kb in range(max(n_a, n_b)):
                    if kb < n_a:
                        emit_A_tile(ci_a, h_a, b_a, qc_a, kb, panel_a)
                    if kb < n_b:
                        dg = kb - qc_b * dpb
                        off = 128 * dg if dg > 0 else 0
                        for half, zp in ((0, zps[0]), (1, zps[1])):
                            nc.tensor.matmul(zp[:, off:],
                                             v_sb[:, b_b, kb, h_b, half, :],
                                             panel_b[:, kb, off:],
                                             start=(kb == 0),
                                             stop=(kb == n_b - 1),
                                             skip_group_check=True)
                if b_args is None:
                    return
                zta, ztb = zps
                # drain PSUM fast: s row + both z halves to SBUF (3 quick
                # DVE ops) so the z banks recycle ahead of the ship below
                ssb = work.tile([128, QC], F32, tag="ssb", bufs=3,
                                name=f"ssb{ci_b}")
                nc.vector.tensor_copy(ssb[64:65, :], zta[64:65, :])
                zsb = []
                for half, zp in ((0, zta), (1, ztb)):
                    zs = work.tile([64, QC], BF16, tag=f"zsb{half}", bufs=3,
                                   name=f"zsb{ci_b}_{half}")
                    nc.vector.tensor_copy(zs, zp[0:64, :])
                    zsb.append(zs)
                chain[ci_b] = (ssb, zsb, h_b, b_b, qc_b)

            def ship(ci):
                # one chunk behind its AV pass: broadcast s to all partitions
                # with a rank-1 PE matmul (deterministically paced by the PE
                # stream, no DRAM bounce, no gpsimd hop), then 1/s and scale
                ssb, zsb, h, b, qc = chain.pop(ci)
                rb_ps = ps.tile([128, QC], F32, tag="z", bufs=3,
                                name=f"rbp{ci}")
                nc.tensor.matmul(rb_ps, onesrow_sb[64:65, :], ssb[64:65, :],
                                 start=True, stop=True)
                rb = work.tile([128, QC], F32, tag="rb", bufs=3, name=f"rb{ci}")
                nc.vector.reciprocal_approx_fast(out=rb, in_=rb_ps)
                shard = b * n_qc + qc
                for half, zs in ((0, zsb[0]), (1, zsb[1])):
                    nc.vector.tensor_mul(zs, zs, rb[0:64, :])
                    nc.sync.dma_start(
                        out=a2a_in[h][shard][half * 64:(half + 1) * 64, :],
                        in_=zs)

            zf_sb = stat.tile([128, n_heads, rows], BF16, tag="wvzf")

            def emit_cc(h):
                nc.gpsimd.collective_compute(
                    "AllToAll", mybir.AluOpType.bypass, replica_groups=rg,
                    ins=[a2a_in[h].opt()], outs=[a2a_out[h].opt()])

            def load_zf(h):
                # scalar HWDGE queue: idle after phase 2's exps. Explicitly
                # pinned after the final exp so the scheduler cannot hoist the
                # collective-completion wait into the middle of phase 2.
                for i in range(n_cores):
                    d = nc.scalar.dma_start(out=zf_sb[:, i * hpc + h, :],
                                            in_=a2a_out[h][i])
                    if last_exp[0] is not None:
                        add_dep_helper(d.ins, last_exp[0].ins,
                                       reason="zf load after phase-2 exps")

            per_h = len(chunks) // hpc

            def do_ship(ci, h):
                ship(ci)
                if ci % per_h == per_h - 1:
                    emit_cc(h)

            prev = None
            for ci, (h, b, qc) in enumerate(chunks):
                if ci >= 2:
                    do_ship(ci - 2, chunks[ci - 2][0])
                emit_iter((ci, h, b, qc), prev)
                prev = (ci, h, b, qc)
            n = len(chunks)
            if n >= 2:
                do_ship(n - 2, chunks[n - 2][0])
            emit_iter(None, prev)
            do_ship(n - 1, chunks[n - 1][0])
            # scalar-queue order at phase-2 end: zf evens (cc0 long done),
            # then W_O chunks 0/1 into the released qt/kt slots, then zf odds
            # (which wait on cc1). All pinned after the exps so the scheduler
            # cannot hoist any of these waits into phase 2.
            load_zf(0)
            for mc, tg in zip(range(min(2, n_mc)), ("qt_sb", "kt_sb")):
                t = stat.tile([128, n_heads, MC], BF16, tag=tg, name=f"wo{mc}")
                d = nc.scalar.dma_start(out=t,
                                        in_=wo_d[:, :, mc * MC:(mc + 1) * MC])
                if last_exp[0] is not None:
                    add_dep_helper(d.ins, last_exp[0].ins,
                                   reason="wo load after phase-2 exps")
                wo_tiles[mc] = t
            for h in range(1, hpc):
                load_zf(h)
            # remaining W_O chunks reuse the idle xt slots, on the gpsimd
            # queue right after the collective triggers
            for mc in range(2, n_mc):
                t = xin.tile([128, n_heads, MC], BF16, tag="xt", name=f"wo{mc}")
                nc.gpsimd.dma_start(
                    out=t, in_=wo_d[:, :, mc * MC:(mc + 1) * MC])
                wo_tiles[mc] = t

            # ---- phase 3: output projection, two passes ----
            # Pass E runs the first-A2A heads for ALL output tiles (~33us of
            # PE work gated only on cc0) staged to SBUF in bf16; pass O adds
            # the remaining heads once cc1's zf tiles have long arrived.
            heads_of = {h: [i * hpc + h for i in range(n_cores)]
                        for h in range(hpc)}
            late_heads = [g for h in range(1, hpc) for g in heads_of[h]]
            osbe = stat.tile([128, n_mc * n_pb, MC], BF16)
            tiles3 = [(mc, pb) for mc in range(n_mc) for pb in range(n_pb)]
            for n, (mc, pb) in enumerate(tiles3):
                tg, bf = (("A", 2), ("st", 3))[n % 2]
                pp = ps.tile([128, MC], F32, tag=tg, bufs=bf,
                             name=f"pse{mc}_{pb}")
                for j, g in enumerate(heads_of[0]):
                    nc.tensor.matmul(pp, zf_sb[:, g, pb * 128:(pb + 1) * 128],
                                     wo_tiles[mc][:, g, :],
                                     start=(j == 0),
                                     stop=(j == len(heads_of[0]) - 1))
                nc.vector.tensor_copy(osbe[:, n, :], pp)
            for n, (mc, pb) in enumerate(tiles3):
                tg, bf = (("A", 2), ("st", 3))[n % 2]
                pp = ps.tile([128, MC], F32, tag=tg, bufs=bf,
                             name=f"pso{mc}_{pb}")
                for j, g in enumerate(late_heads):
                    nc.tensor.matmul(pp, zf_sb[:, g, pb * 128:(pb + 1) * 128],
                                     wo_tiles[mc][:, g, :],
                                     start=(j == 0),
                                     stop=(j == len(late_heads) - 1))
                osb = work.tile([128, MC], F32, tag="osb", bufs=2,
                                name=f"osb{mc}_{pb}")
                nc.vector.tensor_add(osb, pp, osbe[:, n, :])
                ml = slice(mc * MC, (mc + 1) * MC)
                nc.sync.dma_start(out=out_d[pb * 128:(pb + 1) * 128, ml],
                                  in_=osb)

    nc.compile()
    return nc


def make_in_maps(inputs, cfg=FULL):
    c = _derived(cfg)
    hpc, QC = c["hpc"], c["qc_size"]
    n_mb, n_dg = c["n_mb"], c["n_dg"]
    d_model, seq, batch = c["d_model"], c["seq"], c["batch"]
    residual = np.asarray(inputs["residual"], np.float32)
    W_Q = np.asarray(inputs["W_Q"], np.float32)
    W_K = np.asarray(inputs["W_K"], np.float32)
    W_V = np.asarray(inputs["W_V"], np.float32)
    W_O = np.asarray(inputs["W_O"], np.float32)
    b_Q = np.asarray(inputs["b_Q"], np.float32)
    b_K = np.asarray(inputs["b_K"], np.float32)
    b_V = np.asarray(inputs["b_V"], np.float32)
    scale = 1.0 / ATTN_SCALE

    # X^T packed per q-chunk [128, batch, n_qc, n_mb, QC]:
    # [p, b, qc, mb, s'] = residual[b, qc*QC+s', mb*128+p]
    n_qc = c["n_qc"]
    xt = np.ascontiguousarray(
        residual.reshape(batch, n_qc, QC, n_mb, 128).transpose(4, 0, 1, 3, 2)
    ).astype(NP_BF16)
    # W_O packed [128, n_heads, d_model]
    wo = np.ascontiguousarray(
        W_O.transpose(1, 0, 2)).astype(NP_BF16)
    # causal {0,1} masks packed [128, n_dg, QC]
    masks = np.zeros((128, n_dg, QC), np.float32)
    pk = np.arange(128)[:, None]
    fq = np.arange(QC)[None, :]
    for dg in range(n_dg):
        masks[:, dg, :] = (fq >= pk + 128 * dg).astype(np.float32)
    masks = masks.astype(NP_BF16)

    in_maps = []
    for core in range(c["n_cores"]):
        hs = slice(core * hpc, (core + 1) * hpc)
        # [128, hpc, n_mb, 128]: [p, h, mb, d] = W[h, mb*128+p, d]
        wq = np.ascontiguousarray(
            (W_Q[hs] * scale).reshape(hpc, n_mb, 128, 128).transpose(2, 0, 1, 3)
        ).astype(NP_BF16)
        wk = np.ascontiguousarray(
            W_K[hs].reshape(hpc, n_mb, 128, 128).transpose(2, 0, 1, 3)
        ).astype(NP_BF16)
        # [128, n_mb, hpc*128]: [p, mb, (h d)] = W_V[h, mb*128+p, d]
        wv = np.ascontiguousarray(
            W_V[hs].reshape(hpc, n_mb, 128, 128).transpose(2, 1, 0, 3)
            .reshape(128, n_mb, hpc * 128)).astype(NP_BF16)
        bq = np.ascontiguousarray((b_Q[hs] * scale).T).astype(np.float32)
        bk = np.ascontiguousarray(b_K[hs].T).astype(np.float32)
        bv = np.ascontiguousarray(b_V[hs].reshape(hpc * 128)).astype(np.float32)
        in_maps.append({
            "xt": xt, "wq": wq, "wk": wk, "wv": wv, "wo": wo,
            "bq": bq, "bk": bk, "bv": bv, "mk": masks,
        })
    return in_maps


def assemble_output(inputs, shards, cfg=FULL):
    c = _derived(cfg)
    residual = np.asarray(inputs["residual"], np.float32)
    b_O = np.asarray(inputs["b_O"], np.float32)
    out = np.concatenate([np.asarray(s, np.float32) for s in shards], axis=0)
    out = out.reshape(c["batch"], c["seq"], c["d_model"]) + b_O
    return residual, out.astype(np.float32)


_NC_CACHE = {}


def _get_nc():
    if "nc" not in _NC_CACHE:
        _NC_CACHE["nc"] = build_graph(FULL)
    return _NC_CACHE["nc"]


def run(inputs, trace=False):
    nc = _get_nc()
    in_maps = make_in_maps(inputs, FULL)
    try:
        res = run_bass_kernel_spmd(nc, in_maps, list(range(FULL["n_cores"])),
                                   trace=trace)
    except Exception:
        # a previous bad run can leave the remote device wedged for one
        # attempt; give it a moment and retry once
        import time
        time.sleep(60)
        res = run_bass_kernel_spmd(nc, in_maps, list(range(FULL["n_cores"])),
                                   trace=trace)
    shards = [res.results[i]["out"] for i in range(FULL["n_cores"])]
    residual, out = assemble_output(inputs, shards, FULL)
    return (residual, out), res


def kernel(**inputs):
    (residual, out), _ = run(inputs, trace=False)
    return (residual, out)
